# revision 1
# baseline (speedup 1.0000x reference)
"""Causal self-attention Trainium2 kernel (8 NeuronCores, SPMD).

Problem (hardcoded): B=2, T=2048, C=1024, H=16, D=64.
  qkv = x @ W_attn + b_attn ; causal softmax attention ; y @ W_out + b_out

Sharding: core c handles batch b = c//4 and head group g = c%4 (4 heads,
256 channels). Each core computes its heads' attention output and a
partial out-projection [2048, 1024]; the host sums the 4 partials per
batch and adds b_out.

All matmuls run as float32r (full-rate fp32, ~1e-4 rel err).
Layouts are chosen so no on-device transposes are needed:
  - x arrives transposed ([C, T]) from the host.
  - Q^T/K^T are produced directly in [D, T] (heads paired on 128
    partitions) by using W as the stationary operand.
  - scores are computed transposed (S^T[t, q]) so the softmax sum comes
    free from an appended ones-column on V ([V|1] trick), and exp(S^T)
    blocks feed att@V as the moving operand directly.
  - att@V produces O^T[d, q]; normalization multiplies by the
    broadcast reciprocal of the rowsum row (K=1 matmul broadcast).
"""

import sys

if "/opt/trn_rl_repo" not in sys.path:
    sys.path.insert(0, "/opt/trn_rl_repo")

import numpy as np

import concourse.bass as bass
import concourse.mybir as mybir
import concourse.tile as tile
from concourse import bacc, bass_utils

F32 = mybir.dt.float32
F32R = mybir.dt.float32r
MULT = mybir.AluOpType.mult
EXP = mybir.ActivationFunctionType.Exp

B, T, C = 2, 2048, 1024
H, D = 16, 64
HPC = 4          # heads per core
GC = HPC * D     # channels per core's head group (256)
NT = T // 128    # 16 t-tiles
NK = C // 128    # 8 contraction tiles
QCH = 512        # q-chunk width
SCALE = float(1.0 / np.sqrt(D))

_CACHE = {}


def _build(iters=1, phases=3, ablate=None):
    nc = bacc.Bacc("TRN2", target_bir_lowering=False, debug=False,
                   enable_asserts=False, num_devices=8)
    xt_d = nc.dram_tensor("xt", [C, T], F32, kind="ExternalInput").ap()
    wq_d = nc.dram_tensor("wq", [C, GC], F32, kind="ExternalInput").ap()
    wk_d = nc.dram_tensor("wk", [C, GC], F32, kind="ExternalInput").ap()
    wv_d = nc.dram_tensor("wv", [C, GC], F32, kind="ExternalInput").ap()
    bqk_d = nc.dram_tensor("bqk", [128, 4], F32, kind="ExternalInput").ap()
    bv_d = nc.dram_tensor("bv", [128, GC], F32, kind="ExternalInput").ap()
    wo_d = nc.dram_tensor("wo", [GC, C], F32, kind="ExternalInput").ap()
    mask_d = nc.dram_tensor("mask", [128, 128], F32, kind="ExternalInput").ap()
    ones1_d = nc.dram_tensor("ones1", [1, D], F32, kind="ExternalInput").ap()
    onesv_d = nc.dram_tensor("onesv", [128, NT, HPC, 1], F32, kind="ExternalInput").ap()
    y_d = nc.dram_tensor("y", [T, C], F32, kind="ExternalOutput").ap()

    import contextlib

    with tile.TileContext(nc) as tc, nc.allow_low_precision(reason="f32r is 32-bit"):
        loop_ctx = tc.For_i(0, iters, 1) if iters > 1 else contextlib.nullcontext()
        with loop_ctx, tc.tile_pool(name="persist", bufs=1) as sb:
            xt = sb.tile([128, NK, T], F32R)
            wq = sb.tile([128, NK, GC], F32R)
            wk = sb.tile([128, NK, GC], F32R)
            wv = sb.tile([128, NK, GC], F32R)
            bqk = sb.tile([128, 4], F32)
            bv = sb.tile([128, GC], F32)
            wo = sb.tile([128, GC // 128, C], F32R)
            mask = sb.tile([128, 128], F32R)
            ones1 = sb.tile([1, D], F32R)
            qt = [sb.tile([128, T], F32R, name=f"qt{i}") for i in range(2)]
            kt = [sb.tile([128, T], F32R, name=f"kt{i}") for i in range(2)]
            vs = sb.tile([128, NT, HPC, D + 1], F32R)
            ot = [sb.tile([128, T], F32R, name=f"ot{i}") for i in range(2)]

            nc.sync.dma_start(out=wq, in_=wq_d.rearrange("(k p) d -> p k d", p=128).bitcast(F32R))
            nc.sync.dma_start(out=wk, in_=wk_d.rearrange("(k p) d -> p k d", p=128).bitcast(F32R))
            xt_src = xt_d.rearrange("(k p) t -> p k t", p=128).bitcast(F32R)
            for k in range(NK):
                nc.sync.dma_start(out=xt[:, k, :], in_=xt_src[:, k, :])
            nc.sync.dma_start(out=wv, in_=wv_d.rearrange("(k p) d -> p k d", p=128).bitcast(F32R))
            nc.sync.dma_start(out=bqk, in_=bqk_d)
            nc.sync.dma_start(out=bv, in_=bv_d)
            nc.sync.dma_start(out=wo, in_=wo_d.rearrange("(k p) e -> p k e", p=128).bitcast(F32R))
            nc.sync.dma_start(out=mask, in_=mask_d.bitcast(F32R))
            nc.sync.dma_start(out=ones1, in_=ones1_d.bitcast(F32R))
            nc.sync.dma_start(out=vs[:, :, :, D:D + 1], in_=onesv_d.bitcast(F32R))

            # ---- Phase 1 + 2: QKV projections and attention, overlapped ----
            def proj_qkt_chunk(ps1, half, n):
                for w_sb, b_col, dst in ((wq, half, qt[half]), (wk, 2 + half, kt[half])):
                    acc = ps1.tile([128, QCH], F32, tag="acc")
                    for k in range(NK):
                        nc.tensor.matmul(
                            acc,
                            lhsT=w_sb[:, k, 128 * half:128 * (half + 1)],
                            rhs=xt[:, k, QCH * n:QCH * (n + 1)],
                            start=(k == 0), stop=(k == NK - 1))
                    nc.vector.tensor_scalar_add(
                        out=dst[:, QCH * n:QCH * (n + 1)], in0=acc,
                        scalar1=bqk[:, b_col:b_col + 1])

            def proj_qt_kt(ps1, half):
                for n in range(T // QCH):
                    proj_qkt_chunk(ps1, half, n)

            # Attention chunk, processed jointly for a HEAD PAIR: both heads'
            # ST matmuls for strip j go back-to-back (they sit on disjoint
            # 64-row PE row-groups, so they run concurrently, and they write
            # different PSUM banks of one shared two-bank tile). One ACT exp
            # covers both heads' strips, amortizing ACT's 352-cycle per-op
            # overhead. The att@V matmuls lag the ST/exp stream by one strip
            # so the in-order PE stream doesn't stall on exp.
            def attn_strips_pair(pools, hp, m0, pending):
                # `pending` carries un-emitted att@V work across chunk
                # boundaries so the PE stream never drains at a chunk end.
                pt_pool, nrm_pool, ps_st, ps_ot, ps_bc = pools
                half = hp
                heads = (2 * hp, 2 * hp + 1)
                q0 = 128 * m0
                ots = [ps_ot.tile([D + 1, QCH], F32, tag="ot", name=f"psum_ot{i}")
                       for i in range(2)]

                def strip_w(j):
                    return QCH - ((j - m0) * 128 if j > m0 else 0)

                for j in range(m0 + QCH // 128):
                    w = strip_w(j)
                    psum_st = ps_st.tile([128, 2 * QCH], F32, tag="st",
                                         name="psum_st")
                    for idx, h in enumerate(heads):
                        poff = 64 * (h % 2)
                        nc.tensor.matmul(
                            psum_st[:, QCH * idx:QCH * idx + w],
                            lhsT=kt[half][poff:poff + D, 128 * j:128 * (j + 1)],
                            rhs=qt[half][poff:poff + D, q0 + QCH - w:q0 + QCH],
                            start=True, stop=True)
                    # one exp over both heads' strips (covers any dead gap
                    # between them; those columns are never read downstream)
                    span = QCH + w
                    pt = pt_pool.tile([128, 2 * QCH], F32R, tag="pt", name="pt")
                    if ablate == "noexp":
                        nc.vector.tensor_copy(out=pt[:, 0:span], in_=psum_st[:, 0:span])
                    else:
                        nc.scalar.activation(out=pt[:, 0:span], in_=psum_st[:, 0:span],
                                             func=EXP, scale=SCALE)
                    if j >= m0:
                        for idx in range(2):
                            nc.vector.tensor_tensor(
                                out=pt[:, QCH * idx:QCH * idx + 128],
                                in0=pt[:, QCH * idx:QCH * idx + 128],
                                in1=mask, op=MULT)
                    pending.append((hp, j, pt, ots, m0))
                    if len(pending) > 2:
                        emit_attv(*pending.pop(0))
                return [(ots[0], half, 0, q0), (ots[1], half, 64, q0)]

            def emit_attv(hp, j, pt, ots_, m0_):
                # One full-width matmul per head: `stop` is sim-only, so no
                # need to split out the diagonal region (an N=128 matmul
                # would run at 4x cost under f32r).
                sb_off = (j - m0_) * 128 if j > m0_ else 0
                w = QCH - sb_off
                last = (j == m0_ + QCH // 128 - 1)
                for idx, h in enumerate((2 * hp, 2 * hp + 1)):
                    off = QCH * idx
                    nc.tensor.matmul(
                        ots_[idx][:, sb_off:QCH],
                        lhsT=vs[:, j, h, :], rhs=pt[:, off:off + w],
                        start=(j == 0), stop=last, skip_group_check=True)

            def attn_flush(pending):
                for args in pending:
                    emit_attv(*args)
                pending.clear()

            def attn_norm(pools, state):
                pt_pool, nrm_pool, ps_st, ps_ot, ps_bc = pools
                psum_ot, half, poff, q0 = state
                if ablate == "nonorm":
                    nc.vector.tensor_copy(out=ot[half][poff:poff + D, q0:q0 + QCH],
                                          in_=psum_ot[0:D, :])
                    return
                rs_recip = nrm_pool.tile([1, QCH], F32R, tag="rs", name="rs_recip")
                nc.vector.reciprocal(out=rs_recip, in_=psum_ot[D:D + 1, :])
                psum_bc = ps_bc.tile([D, QCH], F32, tag="bc", name="psum_bc")
                nc.tensor.matmul(psum_bc, lhsT=ones1, rhs=rs_recip,
                                 start=True, stop=True)
                bc_sb = nrm_pool.tile([D, QCH], F32, tag="bcs", name="bc_sb")
                nc.vector.tensor_copy(out=bc_sb, in_=psum_bc)
                nc.vector.tensor_tensor(
                    out=ot[half][poff:poff + D, q0:q0 + QCH],
                    in0=psum_ot[0:D, :], in1=bc_sb, op=MULT)

            def outproj_block(ps_mm, ystage, m0):
                for i in range(m0, m0 + QCH // 128):
                    for n in range(C // QCH):
                        acc = ps_mm.tile([128, QCH], F32, tag="acc", name="acc")
                        for half in range(2):
                            nc.tensor.matmul(
                                acc,
                                lhsT=ot[half][:, 128 * i:128 * (i + 1)],
                                rhs=wo[:, half, QCH * n:QCH * (n + 1)],
                                start=(half == 0), stop=(half == 1))
                        yt = ystage.tile([128, QCH], F32, tag="yt", name="yt")
                        nc.vector.tensor_copy(out=yt, in_=acc)
                        nc.sync.dma_start(
                            out=y_d[128 * i:128 * (i + 1), QCH * n:QCH * (n + 1)],
                            in_=yt)

            with tc.tile_pool(name="ps_mm", bufs=1, space="PSUM") as ps_mm, \
                 tc.tile_pool(name="ystage", bufs=2) as ystage:
                if phases < 2:
                    proj_qt_kt(ps_mm, 0)

                def vproj(j):
                    accv = ps_mm.tile([128, GC], F32, tag="acc", name="accv")
                    for k in range(NK):
                        nc.tensor.matmul(
                            accv,
                            lhsT=xt[:, k, 128 * j:128 * (j + 1)],
                            rhs=wv[:, k, :],
                            start=(k == 0), stop=(k == NK - 1))
                    nc.vector.tensor_tensor(
                        out=vs[:, j, :, 0:D],
                        in0=accv.rearrange("p (h d) -> p h d", h=HPC),
                        in1=bv.rearrange("p (h d) -> p h d", h=HPC),
                        op=mybir.AluOpType.add)

                if phases < 2:
                    for j in range(NT):
                        vproj(j)
                with tc.tile_pool(name="pt_pool", bufs=4) as pt_pool, \
                     tc.tile_pool(name="nrm_pool", bufs=2) as nrm_pool, \
                     tc.tile_pool(name="ps_st", bufs=2, space="PSUM") as ps_st, \
                     tc.tile_pool(name="ps_ot", bufs=2, space="PSUM") as ps_ot, \
                     tc.tile_pool(name="ps_bc", bufs=1, space="PSUM") as ps_bc:
                    pools = (pt_pool, nrm_pool, ps_st, ps_ot, ps_bc)
                    # pair 0 attention (emitted before half-1 proj so it
                    # takes PE priority as soon as deps are ready; half-1
                    # proj fills PE gaps while ACT/DVE work on pair 0).
                    # Each task's normalizes are deferred past the next
                    # task's strips to keep the PE stream stall-free.
                    if phases >= 2:
                        prev = None
                        pending = []
                        for m0 in range(0, NT, QCH // 128):
                            # Chunk m0 needs exactly qt/kt column-chunk m0/4
                            # and V tiles m0..m0+3; emitting them here keeps
                            # PE dense while letting ACT start exp almost
                            # immediately instead of idling through the
                            # whole projection.
                            proj_qkt_chunk(ps_mm, 0, m0 // (QCH // 128))
                            for j in range(m0, m0 + QCH // 128):
                                vproj(j)
                            states = attn_strips_pair(pools, 0, m0, pending)
                            # half-1 projection chunks ride along as PE
                            # filler while ACT chews on pair-0 exp work
                            proj_qkt_chunk(ps_mm, 1, m0 // (QCH // 128))
                            if prev is not None:
                                for st_ in prev:
                                    attn_norm(pools, st_)
                            prev = states
                        attn_flush(pending)
                        for st_ in prev:
                            attn_norm(pools, st_)
                    if phases < 2:
                        proj_qt_kt(ps_mm, 1)
                    if phases >= 2:
                        prev = None
                        prev_m0 = None
                        pending = []
                        for m0 in range(0, NT, QCH // 128):
                            states = attn_strips_pair(pools, 1, m0, pending)
                            if prev is not None:
                                for st_ in prev:
                                    attn_norm(pools, st_)
                            if phases >= 3 and prev_m0 is not None:
                                outproj_block(ps_mm, ystage, prev_m0)
                            prev = states
                            prev_m0 = m0
                        attn_flush(pending)
                        for st_ in prev:
                            attn_norm(pools, st_)
                        if phases >= 3:
                            outproj_block(ps_mm, ystage, prev_m0)
    nc.compile()
    return nc


def _get_nc():
    if "nc" not in _CACHE:
        _CACHE["nc"] = _build()
    return _CACHE["nc"]


def make_in_maps(x, W_attn, b_attn, W_out):
    """Per-core input dicts for the SPMD kernel."""
    x = np.asarray(x, dtype=np.float32)
    W_attn = np.asarray(W_attn, dtype=np.float32)
    b_attn = np.asarray(b_attn, dtype=np.float32)
    W_out = np.asarray(W_out, dtype=np.float32)
    mask = np.triu(np.ones((128, 128), np.float32))
    ones1 = np.ones((1, D), np.float32)
    onesv = np.ones((128, NT, HPC, 1), np.float32)
    in_maps = []
    for c in range(8):
        b, g = divmod(c, 4)
        sl = slice(g * GC, (g + 1) * GC)
        bq = b_attn[0 * C:][sl].reshape(2, 128).T          # [128, 2] halves
        bk = b_attn[1 * C:][sl].reshape(2, 128).T
        bqk = np.ascontiguousarray(
            np.stack([bq[:, 0], bq[:, 1], bk[:, 0], bk[:, 1]], axis=1))
        bv = np.tile(b_attn[2 * C:][sl][None, :], (128, 1))
        in_maps.append({
            "xt": np.ascontiguousarray(x[b].T),
            "wq": np.ascontiguousarray(W_attn[:, 0 * C:][:, sl]),
            "wk": np.ascontiguousarray(W_attn[:, 1 * C:][:, sl]),
            "wv": np.ascontiguousarray(W_attn[:, 2 * C:][:, sl]),
            "bqk": bqk,
            "bv": np.ascontiguousarray(bv),
            "wo": np.ascontiguousarray(W_out[sl, :]),
            "mask": mask,
            "ones1": ones1,
            "onesv": onesv,
        })
    return in_maps


def assemble(results, b_out):
    """Sum per-core partials into the full [B, T, C] output."""
    y = np.zeros((B, T, C), np.float32)
    for c in range(8):
        y[c // 4] += results[c]["y"]
    y += np.asarray(b_out, dtype=np.float32)[None, None, :]
    return y


def kernel(x, W_attn, b_attn, W_out, b_out):
    nc = _get_nc()
    in_maps = make_in_maps(x, W_attn, b_attn, W_out)
    res = bass_utils.run_bass_kernel_spmd(nc, in_maps, core_ids=list(range(8)))
    return assemble(res.results, b_out)



# revision 4
# speedup vs baseline: 1.3820x; 1.3820x over previous
"""Causal self-attention Trainium2 kernel (8 NeuronCores, SPMD).

Problem (hardcoded): B=2, T=2048, C=1024, H=16, D=64.
  qkv = x @ W_attn + b_attn ; causal softmax attention ; y @ W_out + b_out

Sharding: core c handles batch b = c//4 and head group g = c%4 (4 heads,
256 channels). Each core computes its heads' attention output and a
partial out-projection [2048, 1024]; the host sums the 4 partials per
batch and adds b_out.

All matmuls run as float32r (full-rate fp32, ~1e-4 rel err).
Layouts are chosen so no on-device transposes are needed:
  - x arrives transposed ([C, T]) from the host.
  - Q^T/K^T are produced directly in [D, T] (heads paired on 128
    partitions) by using W as the stationary operand.
  - scores are computed transposed (S^T[t, q]) so the softmax sum comes
    free from an appended ones-column on V ([V|1] trick), and exp(S^T)
    blocks feed att@V as the moving operand directly.
  - att@V produces O^T[d, q]; normalization multiplies by the
    broadcast reciprocal of the rowsum row (K=1 matmul broadcast).
"""

import sys

if "/opt/trn_rl_repo" not in sys.path:
    sys.path.insert(0, "/opt/trn_rl_repo")

import numpy as np
import ml_dtypes

BF = ml_dtypes.bfloat16

import concourse.bass as bass
import concourse.mybir as mybir
import concourse.tile as tile
from concourse import bacc, bass_utils

F32 = mybir.dt.float32
F32R = mybir.dt.float32r
BF16 = mybir.dt.bfloat16
MULT = mybir.AluOpType.mult
EXP = mybir.ActivationFunctionType.Exp

B, T, C = 2, 2048, 1024
H, D = 16, 64
HPC = 4          # heads per core
GC = HPC * D     # channels per core's head group (256)
NT = T // 128    # 16 t-tiles
NK = C // 128    # 8 contraction tiles
QCH = 512        # q-chunk width
SCALE = float(1.0 / np.sqrt(D))

_CACHE = {}


def _build(iters=1, phases=3, ablate=None):
    nc = bacc.Bacc("TRN2", target_bir_lowering=False, debug=False,
                   enable_asserts=False, num_devices=8)
    xt_d = nc.dram_tensor("xt", [C, T], BF16, kind="ExternalInput").ap()
    wq_d = nc.dram_tensor("wq", [C, GC], BF16, kind="ExternalInput").ap()
    wk_d = nc.dram_tensor("wk", [C, GC], BF16, kind="ExternalInput").ap()
    wv_d = nc.dram_tensor("wv", [C, GC], BF16, kind="ExternalInput").ap()
    bqk_d = nc.dram_tensor("bqk", [128, 4], F32, kind="ExternalInput").ap()
    bv_d = nc.dram_tensor("bv", [128, GC], F32, kind="ExternalInput").ap()
    wo_d = nc.dram_tensor("wo", [GC, C], BF16, kind="ExternalInput").ap()
    mask_d = nc.dram_tensor("mask", [128, 128], BF16, kind="ExternalInput").ap()
    ones1_d = nc.dram_tensor("ones1", [1, D], F32, kind="ExternalInput").ap()
    onesv_d = nc.dram_tensor("onesv", [128, NT, HPC, 1], BF16, kind="ExternalInput").ap()
    y_d = nc.dram_tensor("y", [T, C], F32, kind="ExternalOutput").ap()

    import contextlib

    with tile.TileContext(nc) as tc, nc.allow_low_precision(reason="f32r is 32-bit"):
        loop_ctx = tc.For_i(0, iters, 1) if iters > 1 else contextlib.nullcontext()
        with loop_ctx, tc.tile_pool(name="persist", bufs=1) as sb:
            xt = sb.tile([128, NK, T], BF16)
            wq = sb.tile([128, NK, GC], BF16)
            wk = sb.tile([128, NK, GC], BF16)
            wv = sb.tile([128, NK, GC], BF16)
            bqk = sb.tile([128, 4], F32)
            bv = sb.tile([128, GC], F32)
            wo = sb.tile([128, GC // 128, C], BF16)
            mask = sb.tile([128, 128], BF16)
            ones1 = sb.tile([1, D], F32R)
            qt = [sb.tile([128, T], BF16, name=f"qt{i}") for i in range(2)]
            kt = [sb.tile([128, T], BF16, name=f"kt{i}") for i in range(2)]
            vs = sb.tile([128, NT, HPC, D + 1], BF16)
            ot = [sb.tile([128, T], BF16, name=f"ot{i}") for i in range(2)]

            nc.sync.dma_start(out=wq, in_=wq_d.rearrange("(k p) d -> p k d", p=128))
            nc.sync.dma_start(out=wk, in_=wk_d.rearrange("(k p) d -> p k d", p=128))
            xt_src = xt_d.rearrange("(k p) t -> p k t", p=128)
            for k in range(NK):
                nc.sync.dma_start(out=xt[:, k, :], in_=xt_src[:, k, :])
            nc.sync.dma_start(out=wv, in_=wv_d.rearrange("(k p) d -> p k d", p=128))
            nc.sync.dma_start(out=bqk, in_=bqk_d)
            nc.sync.dma_start(out=bv, in_=bv_d)
            nc.sync.dma_start(out=wo, in_=wo_d.rearrange("(k p) e -> p k e", p=128))
            nc.sync.dma_start(out=mask, in_=mask_d)
            nc.sync.dma_start(out=ones1, in_=ones1_d.bitcast(F32R))
            nc.sync.dma_start(out=vs[:, :, :, D:D + 1], in_=onesv_d)

            # ---- Phase 1 + 2: QKV projections and attention, overlapped ----
            def proj_qkt_chunk(ps1, half, n):
                for w_sb, b_col, dst in ((wq, half, qt[half]), (wk, 2 + half, kt[half])):
                    acc = ps1.tile([128, QCH], F32, tag="acc")
                    for k in range(NK):
                        nc.tensor.matmul(
                            acc,
                            lhsT=w_sb[:, k, 128 * half:128 * (half + 1)],
                            rhs=xt[:, k, QCH * n:QCH * (n + 1)],
                            start=(k == 0), stop=(k == NK - 1))
                    nc.gpsimd.tensor_scalar_add(
                        out=dst[:, QCH * n:QCH * (n + 1)], in0=acc,
                        scalar1=bqk[:, b_col:b_col + 1])

            def proj_qt_kt(ps1, half):
                for n in range(T // QCH):
                    proj_qkt_chunk(ps1, half, n)

            # Attention chunk, processed jointly for a HEAD PAIR: both heads'
            # ST matmuls for strip j go back-to-back (they sit on disjoint
            # 64-row PE row-groups, so they run concurrently, and they write
            # different PSUM banks of one shared two-bank tile). One ACT exp
            # covers both heads' strips, amortizing ACT's 352-cycle per-op
            # overhead. The att@V matmuls lag the ST/exp stream by one strip
            # so the in-order PE stream doesn't stall on exp.
            def attn_strips_pair(pools, hp, m0, pending):
                # `pending` carries un-emitted att@V work across chunk
                # boundaries so the PE stream never drains at a chunk end.
                pt_pool, nrm_pool, ps_st, ps_ot, ps_bc = pools
                half = hp
                heads = (2 * hp, 2 * hp + 1)
                q0 = 128 * m0
                ots = [ps_ot.tile([D + 1, QCH], F32, tag="ot", name=f"psum_ot{i}")
                       for i in range(2)]

                def strip_w(j):
                    return QCH - ((j - m0) * 128 if j > m0 else 0)

                for j in range(m0 + QCH // 128):
                    w = strip_w(j)
                    psum_st = ps_st.tile([128, 2 * QCH], F32, tag="st",
                                         name="psum_st")
                    for idx, h in enumerate(heads):
                        poff = 64 * (h % 2)
                        nc.tensor.matmul(
                            psum_st[:, QCH * idx:QCH * idx + w],
                            lhsT=kt[half][poff:poff + D, 128 * j:128 * (j + 1)],
                            rhs=qt[half][poff:poff + D, q0 + QCH - w:q0 + QCH],
                            start=True, stop=True)
                    # one exp over both heads' strips (covers any dead gap
                    # between them; those columns are never read downstream)
                    span = QCH + w
                    pt = pt_pool.tile([128, 2 * QCH], BF16, tag="pt", name="pt")
                    if ablate == "noexp":
                        nc.vector.tensor_copy(out=pt[:, 0:span], in_=psum_st[:, 0:span])
                    else:
                        nc.scalar.activation(out=pt[:, 0:span], in_=psum_st[:, 0:span],
                                             func=EXP, scale=SCALE)
                    if j >= m0:
                        for idx in range(2):
                            nc.vector.tensor_tensor(
                                out=pt[:, QCH * idx:QCH * idx + 128],
                                in0=pt[:, QCH * idx:QCH * idx + 128],
                                in1=mask, op=MULT)
                    pending.append((hp, j, pt, ots, m0))
                    if len(pending) > 2:
                        emit_attv(*pending.pop(0))
                return [(ots[0], half, 0, q0), (ots[1], half, 64, q0)]

            def emit_attv(hp, j, pt, ots_, m0_):
                # One full-width matmul per head: `stop` is sim-only, so no
                # need to split out the diagonal region (an N=128 matmul
                # would run at 4x cost under f32r).
                sb_off = (j - m0_) * 128 if j > m0_ else 0
                w = QCH - sb_off
                last = (j == m0_ + QCH // 128 - 1)
                for idx, h in enumerate((2 * hp, 2 * hp + 1)):
                    off = QCH * idx
                    nc.tensor.matmul(
                        ots_[idx][:, sb_off:QCH],
                        lhsT=vs[:, j, h, :], rhs=pt[:, off:off + w],
                        start=(j == 0), stop=last, skip_group_check=True)

            def attn_flush(pending):
                for args in pending:
                    emit_attv(*args)
                pending.clear()

            def attn_norm(pools, state):
                pt_pool, nrm_pool, ps_st, ps_ot, ps_bc = pools
                psum_ot, half, poff, q0 = state
                if ablate == "nonorm":
                    nc.vector.tensor_copy(out=ot[half][poff:poff + D, q0:q0 + QCH],
                                          in_=psum_ot[0:D, :])
                    return
                rs_recip = nrm_pool.tile([1, QCH], F32R, tag="rs", name="rs_recip")
                nc.vector.reciprocal(out=rs_recip, in_=psum_ot[D:D + 1, :])
                psum_bc = ps_bc.tile([D, QCH], F32, tag="bc", name="psum_bc")
                nc.tensor.matmul(psum_bc, lhsT=ones1, rhs=rs_recip,
                                 start=True, stop=True)
                bc_sb = nrm_pool.tile([D, QCH], F32, tag="bcs", name="bc_sb")
                nc.gpsimd.tensor_copy(out=bc_sb, in_=psum_bc)
                nc.vector.tensor_tensor(
                    out=ot[half][poff:poff + D, q0:q0 + QCH],
                    in0=psum_ot[0:D, :], in1=bc_sb, op=MULT)

            def outproj_block(ps_mm, ystage, m0):
                for i in range(m0, m0 + QCH // 128):
                    for n in range(C // QCH):
                        acc = ps_mm.tile([128, QCH], F32, tag="acc", name="acc")
                        for half in range(2):
                            nc.tensor.matmul(
                                acc,
                                lhsT=ot[half][:, 128 * i:128 * (i + 1)],
                                rhs=wo[:, half, QCH * n:QCH * (n + 1)],
                                start=(half == 0), stop=(half == 1))
                        yt = ystage.tile([128, QCH], F32, tag="yt", name="yt")
                        nc.gpsimd.tensor_copy(out=yt, in_=acc)
                        nc.sync.dma_start(
                            out=y_d[128 * i:128 * (i + 1), QCH * n:QCH * (n + 1)],
                            in_=yt)

            with tc.tile_pool(name="ps_mm", bufs=1, space="PSUM") as ps_mm, \
                 tc.tile_pool(name="ystage", bufs=2) as ystage:
                if phases < 2:
                    proj_qt_kt(ps_mm, 0)

                def vproj(j):
                    accv = ps_mm.tile([128, GC], F32, tag="acc", name="accv")
                    for k in range(NK):
                        nc.tensor.matmul(
                            accv,
                            lhsT=xt[:, k, 128 * j:128 * (j + 1)],
                            rhs=wv[:, k, :],
                            start=(k == 0), stop=(k == NK - 1))
                    nc.gpsimd.tensor_tensor(
                        out=vs[:, j, :, 0:D],
                        in0=accv.rearrange("p (h d) -> p h d", h=HPC),
                        in1=bv.rearrange("p (h d) -> p h d", h=HPC),
                        op=mybir.AluOpType.add)

                if phases < 2:
                    for j in range(NT):
                        vproj(j)
                with tc.tile_pool(name="pt_pool", bufs=4) as pt_pool, \
                     tc.tile_pool(name="nrm_pool", bufs=2) as nrm_pool, \
                     tc.tile_pool(name="ps_st", bufs=2, space="PSUM") as ps_st, \
                     tc.tile_pool(name="ps_ot", bufs=2, space="PSUM") as ps_ot, \
                     tc.tile_pool(name="ps_bc", bufs=1, space="PSUM") as ps_bc:
                    pools = (pt_pool, nrm_pool, ps_st, ps_ot, ps_bc)
                    # pair 0 attention (emitted before half-1 proj so it
                    # takes PE priority as soon as deps are ready; half-1
                    # proj fills PE gaps while ACT/DVE work on pair 0).
                    # Each task's normalizes are deferred past the next
                    # task's strips to keep the PE stream stall-free.
                    if phases >= 2:
                        prev = None
                        pending = []
                        for m0 in range(0, NT, QCH // 128):
                            # Chunk m0 needs exactly qt/kt column-chunk m0/4
                            # and V tiles m0..m0+3; emitting them here keeps
                            # PE dense while letting ACT start exp almost
                            # immediately instead of idling through the
                            # whole projection.
                            proj_qkt_chunk(ps_mm, 0, m0 // (QCH // 128))
                            for j in range(m0, m0 + QCH // 128):
                                vproj(j)
                            states = attn_strips_pair(pools, 0, m0, pending)
                            # half-1 projection chunks ride along as PE
                            # filler while ACT chews on pair-0 exp work
                            proj_qkt_chunk(ps_mm, 1, m0 // (QCH // 128))
                            if prev is not None:
                                for st_ in prev:
                                    attn_norm(pools, st_)
                            prev = states
                        attn_flush(pending)
                        for st_ in prev:
                            attn_norm(pools, st_)
                    if phases < 2:
                        proj_qt_kt(ps_mm, 1)
                    if phases >= 2:
                        prev = None
                        prev_m0 = None
                        pending = []
                        for m0 in range(0, NT, QCH // 128):
                            states = attn_strips_pair(pools, 1, m0, pending)
                            if prev is not None:
                                for st_ in prev:
                                    attn_norm(pools, st_)
                            if phases >= 3 and prev_m0 is not None:
                                outproj_block(ps_mm, ystage, prev_m0)
                            prev = states
                            prev_m0 = m0
                        attn_flush(pending)
                        for st_ in prev:
                            attn_norm(pools, st_)
                        if phases >= 3:
                            outproj_block(ps_mm, ystage, prev_m0)
    nc.compile()
    return nc


def _get_nc():
    if "nc" not in _CACHE:
        _CACHE["nc"] = _build()
    return _CACHE["nc"]


def make_in_maps(x, W_attn, b_attn, W_out):
    """Per-core input dicts for the SPMD kernel."""
    x = np.asarray(x, dtype=np.float32)
    W_attn = np.asarray(W_attn, dtype=np.float32)
    b_attn = np.asarray(b_attn, dtype=np.float32)
    W_out = np.asarray(W_out, dtype=np.float32)
    mask = np.triu(np.ones((128, 128), np.float32))
    ones1 = np.ones((1, D), np.float32)
    onesv = np.ones((128, NT, HPC, 1), np.float32)
    in_maps = []
    for c in range(8):
        b, g = divmod(c, 4)
        sl = slice(g * GC, (g + 1) * GC)
        bq = b_attn[0 * C:][sl].reshape(2, 128).T          # [128, 2] halves
        bk = b_attn[1 * C:][sl].reshape(2, 128).T
        bqk = np.ascontiguousarray(
            np.stack([bq[:, 0], bq[:, 1], bk[:, 0], bk[:, 1]], axis=1))
        bv = np.tile(b_attn[2 * C:][sl][None, :], (128, 1))
        in_maps.append({
            "xt": np.ascontiguousarray(x[b].T).astype(BF),
            "wq": np.ascontiguousarray(W_attn[:, 0 * C:][:, sl]).astype(BF),
            "wk": np.ascontiguousarray(W_attn[:, 1 * C:][:, sl]).astype(BF),
            "wv": np.ascontiguousarray(W_attn[:, 2 * C:][:, sl]).astype(BF),
            "bqk": bqk,
            "bv": np.ascontiguousarray(bv),
            "wo": np.ascontiguousarray(W_out[sl, :]).astype(BF),
            "mask": mask.astype(BF),
            "ones1": ones1,
            "onesv": onesv.astype(BF),
        })
    return in_maps


def assemble(results, b_out):
    """Sum per-core partials into the full [B, T, C] output."""
    y = np.zeros((B, T, C), np.float32)
    for c in range(8):
        y[c // 4] += results[c]["y"]
    y += np.asarray(b_out, dtype=np.float32)[None, None, :]
    return y


def kernel(x, W_attn, b_attn, W_out, b_out):
    nc = _get_nc()
    in_maps = make_in_maps(x, W_attn, b_attn, W_out)
    res = bass_utils.run_bass_kernel_spmd(nc, in_maps, core_ids=list(range(8)))
    return assemble(res.results, b_out)

